# revision 40
# baseline (speedup 1.0000x reference)
"""Euclidean distance layer on 8 Trainium2 NeuronCores.

Measured 17545-17829ns over 4 runs (mean 17.7us); predecessor kernels
ran 17.85-19.2us, original baseline 19.5-20us.  Run-to-run variance on
identical NEFFs is ~1.3us on this stack.

out[b, o] = || x[b, :] - weight[:, o] ||_2
x: [512, 256] f32, weight: [256, 1024] f32 -> out: [512, 1024] f32

Sharding: tensor-parallel over output features (8 x 128 columns per core).
Per core:  dist^2 = -2 * ( x~@w~_loc - 0.5*||w~_loc||^2 ) + ||x~||^2
with x~, w~ fp16 roundings (f32 accumulation; ~7e-5 rel err vs the
2e-2 gate).

Design (vs the 19.5us predecessor):
  - 2 input DMAs issued in parallel: packed [w|xT] f16 chunks on the
    sync HWDGE queue; x in batch-partition layout (f16) on the
    Activation HWDGE queue, whose desc-gen starts ~0.75us earlier than
    gpsimd's SWDGE (Pool Q7 startup).  4 serial-ish DMAs -> 2.
  - A dummy Sqrt FIRST on ACT makes the compiler load the
    sqrt_and_others table (square AND sqrt) ONCE, overlapped with the
    input-DMA wait; the predecessor paid 2x1283ns in the critical path.
  - One PSUM bank per batch tile (legal per-bank start/stop groups);
    PE order fills the wait for the DVE w-norm broadcast with the m0+m1
    mains and the w-reduce, then folds land per tile so each ACT sqrt
    starts as early as possible.
  - ||x||^2: tiles 0/1 via ACT Square+accum, tiles 2/3 via DVE
    mul+reduce; DVE does the PSUM->SBUF w-norm broadcast before its
    x-squares so the folds are never hostage to the slower x DMA.

Hardware findings encoded here (each cost a crash/garbage cycle):
  - A manually emitted InstLoadActFuncSet kills the NEFF; only
    compiler-pass-inserted loads work.  Hence the dummy-activation
    trick to steer the set choice.
  - A stop=True matmul followed by more start=False matmuls into the
    same PSUM bank crashes the device (CoreSim accepts it with
    skip_group_check) -> per-tile PSUM banks.
  - Any DMA issued on the Activation queue invalidates the loaded act
    table; the compiler inserts a reload AFTER the DMA issue.  That is
    fine here: both table loads still complete during the input-DMA
    wait, so the fast Act HWDGE queue carries the x DMA anyway.
  - ExternalInput dtype must match the host array dtype exactly; the
    PJRT path binds raw bytes (no cast), unlike CoreSim.
  - Skipping the output-DMA completion waits makes the block-exit
    drains wait instead, ~2.4us SLOWER -> keep semaphore waits.
  - ACT accum activations racing in-flight DMA traffic crash the exec
    unit -> x-square accums gate on all input-DMA semaphores.
  - Measured dead ends: prepared dma_scatter_add+trigger_dma output
    (garbage + 13us slower on HW), skipping output waits (+2.4us, drains
    wait more expensively), hoisting a DMA into the entry bb before the
    init barrier (sequencer reaches its stream ~700ns LATER), PE DVFS
    warm-up via dummy matmuls (capped ~0.2us; ACT chain is the bound),
    splitting the gpsimd DMA (serial desc-gen lands half 2 later).
Raw bacc, manual semaphores.  Host work is layout/dtype prep only.
"""

from contextlib import ExitStack

import numpy as np

B = 512
K = 256
NOUT = 1024
NCORES = 8
NLOC = NOUT // NCORES
P = 128
KT = K // P
MT = B // P

_NC = None


def _build():
    import concourse.bass as bass
    from concourse import bacc, mybir

    f32 = mybir.dt.float32
    f16 = mybir.dt.float16
    Sqrt = mybir.ActivationFunctionType.Sqrt
    Square = mybir.ActivationFunctionType.Square
    ts = bass.ts

    nc = bacc.Bacc(
        "TRN2", target_bir_lowering=False, debug=False, num_devices=NCORES
    )

    IN1W = NLOC + B
    in1 = nc.dram_tensor("in1", [P, KT * IN1W], f16, kind="ExternalInput")
    xnb = nc.dram_tensor("xnb", [P, MT * K], f16, kind="ExternalInput")
    out = nc.dram_tensor("out", [B, NLOC], f32, kind="ExternalOutput")

    with ExitStack() as ctx:
        e = ctx.enter_context
        in1_sb = e(nc.sbuf_tensor("in1s", [P, KT, NLOC + B], f16))
        xnb_sb = e(nc.sbuf_tensor("xnbs", [P, MT, K], f16))
        wlsq = [e(nc.sbuf_tensor(f"wlsq{k}", [P, NLOC], f16)) for k in range(KT)]
        xsq_scrA = e(nc.sbuf_tensor("xsqsA", [P, 2, K], f16))
        xsq_scrD = e(nc.sbuf_tensor("xsqsD", [P, 2, K], f16))
        xsq_colA = e(nc.sbuf_tensor("xsqcA", [P, 2], f32))
        xsq_colD = e(nc.sbuf_tensor("xsqcD", [P, 2], f32))
        neg_q = e(nc.sbuf_tensor("neg_q", [P, 2], f16))
        ones_m = e(nc.sbuf_tensor("ones_m", [2, P], f16))
        wsq_row = e(nc.sbuf_tensor("wsq_row", [2, NLOC], f16))
        out_sb = e(nc.sbuf_tensor("out_sb", [P, MT, NLOC], f32))
        actwarm = e(nc.sbuf_tensor("actwarm", [P, 1], f32))

        ps_w = e(nc.psum_tensor("ps_w", [2, NLOC], f32))
        ps_m = [
            e(nc.psum_tensor(f"ps_m{m}", [P, NLOC], f32)) for m in range(MT)
        ]

        s_in1 = e(nc.semaphore("s_in1"))
        s_xn = e(nc.semaphore("s_xn"))
        s_ini = e(nc.semaphore("s_ini"))
        s_sq = e(nc.semaphore("s_sq"))
        s_mmw = e(nc.semaphore("s_mmw"))
        s_fold = e(nc.semaphore("s_fold"))
        s_brd = e(nc.semaphore("s_brd"))
        s_colD = e(nc.semaphore("s_colD"))
        s_sqrt = e(nc.semaphore("s_sqrt"))
        s_out = e(nc.semaphore("s_out"))
        s_out2 = e(nc.semaphore("s_out2"))

        block = e(nc.Block())

        @block.sync
        def _(sync):
            sync.dma_start(
                out=in1_sb[:, :, :],
                in_=in1[:, :].rearrange("p (c w) -> p c w", c=KT),
            ).then_inc(s_in1, 16)
            sync.wait_ge(s_sqrt, 2)
            sync.dma_start(
                out=out[0 : 2 * P, :].rearrange("(m p) o -> p m o", p=P),
                in_=out_sb[:, 0:2, :],
            ).then_inc(s_out, 16)
            sync.wait_ge(s_out, 16)

        @block.gpsimd
        def _(gpsimd):
            gpsimd.wait_ge(s_xn, 16)

        @block.scalar
        def _(scalar):
            scalar.dma_start(
                out=xnb_sb[:, :, :],
                in_=xnb[:, :].rearrange("p (m k) -> p m k", m=MT),
            ).then_inc(s_xn, 16)
            # early warm: triggers the sqrt_and_others table load(s)
            # (square AND sqrt) while the input DMAs are in flight
            scalar.activation(
                actwarm[:, :], nc.const_aps.aps[(f32, 0.0)], Sqrt
            )
            scalar.wait_ge(s_xn, 16)
            scalar.wait_ge(s_in1, 16)
            for m in range(2):
                scalar.activation(
                    xsq_scrA[:, m, :], xnb_sb[:, m, :], Square,
                    accum_out=xsq_colA[:, m : m + 1],
                )
            scalar.drain()
            for m in range(MT):
                scalar.wait_ge(s_fold, m + 1)
                if m == 2:
                    scalar.wait_ge(s_colD, 1)
                bias = (
                    xsq_colA[:, m : m + 1] if m < 2
                    else xsq_colD[:, m - 2 : m - 1]
                )
                scalar.activation(
                    out_sb[:, m, :], ps_m[m][:, :], Sqrt,
                    bias=bias, scale=-2.0,
                ).then_inc(s_sqrt)
            scalar.wait_ge(s_sqrt, MT)
            scalar.dma_start(
                out=out[2 * P : 4 * P, :].rearrange("(m p) o -> p m o", p=P),
                in_=out_sb[:, 2:4, :],
            ).then_inc(s_out2, 16)
            scalar.wait_ge(s_out2, 16)

        @block.vector
        def _(vector):
            vector.memset(neg_q[:, :], -0.25)
            vector.memset(ones_m[:, :], 1.0).then_inc(s_ini)
            vector.wait_ge(s_in1, 16)
            vector.tensor_mul(
                wlsq[0][:, :], in1_sb[:, 0, 0:NLOC], in1_sb[:, 0, 0:NLOC]
            )
            vector.tensor_mul(
                wlsq[1][:, :], in1_sb[:, 1, 0:NLOC], in1_sb[:, 1, 0:NLOC]
            ).then_inc(s_sq, 2)
            vector.wait_ge(s_mmw, 1)
            vector.tensor_copy(wsq_row[:, :], ps_w[:, :]).then_inc(s_brd)
            vector.wait_ge(s_xn, 16)
            for m in range(2):
                vector.tensor_mul(
                    xsq_scrD[:, m, :], xnb_sb[:, 2 + m, :], xnb_sb[:, 2 + m, :]
                )
            vector.drain()
            for m in range(2):
                inst = vector.tensor_reduce(
                    xsq_colD[:, m : m + 1], xsq_scrD[:, m, :],
                    axis=mybir.AxisListType.X, op=mybir.AluOpType.add,
                )
            inst.then_inc(s_colD)

        @block.tensor
        def _(tensor):
            tensor.wait_ge(s_ini, 1)
            tensor.wait_ge(s_in1, 16)
            # m0+m1 mains and the w-norm reduce fill the wait for the DVE
            # broadcast; fold0/fold1 run back-to-back (shared ones_m lhsT),
            # then each later tile is [mains, fold] so folds land early
            for m in range(2):
                for k in range(KT):
                    tensor.matmul(
                        ps_m[m][:, :],
                        lhsT=in1_sb[:, k, NLOC + m * P : NLOC + (m + 1) * P],
                        rhs=in1_sb[:, k, 0:NLOC],
                        start=(k == 0), stop=False,
                    )
                if m == 0:
                    tensor.wait_ge(s_sq, 2)
                    tensor.matmul(
                        ps_w[:, :], lhsT=neg_q[:, :], rhs=wlsq[0][:, :],
                        start=True, stop=False,
                    )
                    tensor.matmul(
                        ps_w[:, :], lhsT=neg_q[:, :], rhs=wlsq[1][:, :],
                        start=False, stop=True,
                    ).then_inc(s_mmw)
            tensor.wait_ge(s_brd, 1)
            for m in range(2):
                tensor.matmul(
                    ps_m[m][:, :], lhsT=ones_m[:, :], rhs=wsq_row[:, :],
                    start=False, stop=True,
                ).then_inc(s_fold)
            for m in range(2, MT):
                for k in range(KT):
                    tensor.matmul(
                        ps_m[m][:, :],
                        lhsT=in1_sb[:, k, NLOC + m * P : NLOC + (m + 1) * P],
                        rhs=in1_sb[:, k, 0:NLOC],
                        start=(k == 0), stop=False,
                    )
                tensor.matmul(
                    ps_m[m][:, :], lhsT=ones_m[:, :], rhs=wsq_row[:, :],
                    start=False, stop=True,
                ).then_inc(s_fold)

    nc.compile()
    return nc


def _get_nc():
    global _NC
    if _NC is None:
        _NC = _build()
    return _NC


def _make_in_maps(x: np.ndarray, weight: np.ndarray):
    x = np.ascontiguousarray(x.astype(np.float32, copy=False))
    xh = x.astype(np.float16)
    wh = weight.astype(np.float16)
    xt = np.ascontiguousarray(xh.T).reshape(KT, P, B)
    xnb = np.ascontiguousarray(
        xh.reshape(MT, P, K).transpose(1, 0, 2)
    ).reshape(P, MT * K)
    maps = []
    for c in range(NCORES):
        wl = wh[:, c * NLOC : (c + 1) * NLOC].reshape(KT, P, NLOC)
        in1 = np.concatenate([wl, xt], axis=2)
        maps.append(
            {
                "in1": np.ascontiguousarray(
                    in1.transpose(1, 0, 2)
                ).reshape(P, KT * (NLOC + B)),
                "xnb": xnb,
            }
        )
    return maps


def run(x: np.ndarray, weight: np.ndarray, trace: bool = False):
    from concourse.bass_utils import run_bass_kernel_spmd

    nc = _get_nc()
    res = run_bass_kernel_spmd(
        nc, _make_in_maps(x, weight), core_ids=list(range(NCORES)), trace=trace
    )
    full = np.concatenate(
        [res.results[c]["out"] for c in range(NCORES)], axis=1
    )
    return full, res


def kernel(x: np.ndarray, weight: np.ndarray) -> np.ndarray:
    return run(x, weight)[0]
